# revision 7
# baseline (speedup 1.0000x reference)
"""Batched linear-chain CRF forward (log partition) on 8 Trainium2 NeuronCores.

Strategy
--------
Data parallel over batch: B=512 -> 64 sequences per core. The per-sequence
time scan is strictly sequential, so per core we halve the serial chain by
running the CRF *forward* recursion (t=0..511) and *backward* recursion
(t=1023..512) concurrently and meeting in the middle:
    Z[b] = sum_j alpha_m[j, b] * beta_m[j, b]        (forward-backward identity)

The log-semiring matmul is computed in the exp domain so the TensorEngine can
do it as a real matmul:
    fwd:  p_{t}   = (E^T-matmul p_{t-1}) . G_t        E = exp(trans)
    bwd:  q_{t}   = (E-matmul  (q_{t+1} . G_{t+1}))
with G_t[k, b] = exp(feats[b, t, k] - lse_k feats[b, t, :]). The per-(b, t)
logsumexp normalizer keeps every intermediate within e^[-11, +1] for this
data regime (verified range; fp32/bf16 safe) and is added back on the host:
    logZ[b] = log(sum_k pf[k,b] * qb[k,b]) + sum_t lse[b, t]

Per time step on-device: one bf16 128x128x64 matmul (PE) + one elementwise
multiply that simultaneously evacuates PSUM (DVE). G is produced by
xbar transpose-DMA (bf16) + ScalarE exp, in 1 MiB chunks, double buffered.
"""
import os
import sys

import numpy as np

for _p in ("/opt/trn_rl_repo", "/root/.axon_site/_ro/trn_rl_repo"):
    if _p not in sys.path and os.path.isdir(_p):
        sys.path.append(_p)

import ml_dtypes

bf16 = ml_dtypes.bfloat16

B, T, K = 512, 1024, 128
NCORES = 8
BS = B // NCORES          # 64 batch per core
M = T // 2                # meet point: fwd consumes t=0..M-1, bwd t=M..T-1
TC = 64                   # time steps per G chunk (1 MiB bf16 per chunk)

_CACHED = {}


def _build_module():
    import concourse.bass as bass
    import concourse.tile as tile
    from concourse import bacc, mybir
    from contextlib import ExitStack

    fdt = mybir.dt.float32
    hdt = mybir.dt.bfloat16

    nc = bacc.Bacc("TRN2", target_bir_lowering=False, debug=False,
                   num_devices=NCORES)
    g_dram = nc.dram_tensor("g", [T * BS, K], hdt, kind="ExternalInput").ap()
    af_dram = nc.dram_tensor("af", [K, K], hdt, kind="ExternalInput").ap()
    ab_dram = nc.dram_tensor("ab", [K, K], hdt, kind="ExternalInput").ap()
    p0_dram = nc.dram_tensor("p0", [K, BS], hdt, kind="ExternalInput").ap()
    q0_dram = nc.dram_tensor("q0", [K, BS], hdt, kind="ExternalInput").ap()
    pf_dram = nc.dram_tensor("pf", [K, BS], fdt, kind="ExternalOutput").ap()
    qb_dram = nc.dram_tensor("qb", [K, BS], fdt, kind="ExternalOutput").ap()

    EXP = mybir.ActivationFunctionType.Exp
    CW = TC * BS  # chunk width in free elements

    with tile.TileContext(nc) as tc, ExitStack() as ctx:
        consts = ctx.enter_context(tc.tile_pool(name="consts", bufs=1))
        graw_p = ctx.enter_context(tc.tile_pool(name="graw", bufs=2))
        gf_p = ctx.enter_context(tc.tile_pool(name="gf", bufs=2))
        gb_p = ctx.enter_context(tc.tile_pool(name="gb", bufs=2))
        st_p = ctx.enter_context(tc.tile_pool(name="st", bufs=3))
        out_p = ctx.enter_context(tc.tile_pool(name="outs", bufs=1))
        psf_p = ctx.enter_context(tc.tile_pool(name="psf", bufs=2, space="PSUM"))
        psb_p = ctx.enter_context(tc.tile_pool(name="psb", bufs=2, space="PSUM"))

        af_sb = consts.tile([K, K], hdt, tag="af")
        nc.sync.dma_start(af_sb[:], af_dram[:])
        ab_sb = consts.tile([K, K], hdt, tag="ab")
        nc.sync.dma_start(ab_sb[:], ab_dram[:])
        p = consts.tile([K, BS], hdt, tag="p0")
        nc.sync.dma_start(p[:], p0_dram[:])
        q0_sb = consts.tile([K, BS], hdt, tag="q0")
        nc.sync.dma_start(q0_sb[:], q0_dram[:])

        def load_chunk(c, pool, tag):
            """Transpose-DMA chunk c (t in [c*TC, (c+1)*TC)) and exp it.
            Result layout: [K, TC*BS] with free index t_local*BS + b."""
            raw = graw_p.tile([K, CW], hdt, tag="raw" + tag)
            nc.sync.dma_start_transpose(raw[:], g_dram[c * CW:(c + 1) * CW, :])
            g = pool.tile([K, CW], hdt, tag=tag)
            nc.scalar.activation(g[:], raw[:], EXP)
            return g

        gf = gb = None
        q_ps = None  # bwd state lives in PSUM between steps
        for i in range(M):
            tb = T - 1 - i                       # bwd time index
            if i % TC == 0:
                gf = load_chunk(i // TC, gf_p, "gf")
                gb = load_chunk(tb // TC, gb_p, "gb")
            fsl = (i % TC) * BS
            bsl = (tb % TC) * BS

            # bwd: u = q_{t+1} * G_{t+1};  q_t = ab^T-matmul u
            u = st_p.tile([K, BS], hdt, tag="u")
            qin = q0_sb if q_ps is None else q_ps
            nc.vector.tensor_mul(u[:], qin[:], gb[:, bsl:bsl + BS])
            q_ps = psb_p.tile([K, BS], fdt, tag="q")
            nc.tensor.matmul(q_ps[:], ab_sb[:], u[:], start=True, stop=True)

            # fwd: s = af^T-matmul p;  p = s * G_t
            s = psf_p.tile([K, BS], fdt, tag="s")
            nc.tensor.matmul(s[:], af_sb[:], p[:], start=True, stop=True)
            pn = st_p.tile([K, BS], hdt, tag="p")
            nc.vector.tensor_mul(pn[:], s[:], gf[:, fsl:fsl + BS])
            p = pn

        pf_sb = out_p.tile([K, BS], fdt, tag="pf")
        nc.vector.tensor_copy(pf_sb[:], p[:])
        nc.sync.dma_start(pf_dram[:], pf_sb[:])
        qb_sb = out_p.tile([K, BS], fdt, tag="qb")
        nc.vector.tensor_copy(qb_sb[:], q_ps[:])
        nc.sync.dma_start(qb_dram[:], qb_sb[:])

    nc.finalize()
    return nc


def _get_module():
    if "nc" not in _CACHED:
        _CACHED["nc"] = _build_module()
    return _CACHED["nc"]


def kernel(feats: np.ndarray, trans: np.ndarray) -> np.ndarray:
    from concourse.bass_utils import run_bass_kernel_spmd

    feats = np.asarray(feats, np.float32)
    trans = np.asarray(trans, np.float32)

    # per-(b,t) logsumexp over tags: the running normalizer, restored on host
    mx = feats.max(axis=-1)                                   # [B,T]
    lse = mx + np.log(
        np.sum(np.exp(feats - mx[:, :, None], dtype=np.float32), axis=-1)
    )                                                         # [B,T] fp32
    gnorm = feats - lse[:, :, None]                           # [B,T,K]

    E = np.exp(trans, dtype=np.float32)                       # [to, frm]
    af = np.ascontiguousarray(E.T).astype(bf16)               # lhsT fwd [frm,to]
    ab = E.astype(bf16)                                       # lhsT bwd [to,frm]
    p0 = np.zeros((K, BS), np.float32)
    p0[K - 1, :] = 1.0                                        # START one-hot
    p0 = p0.astype(bf16)
    q0 = np.repeat(E[K - 2, :][:, None], BS, axis=1).astype(bf16)  # exp(trans[END,:])

    in_maps = []
    for c in range(NCORES):
        sh = gnorm[c * BS:(c + 1) * BS]                       # [BS,T,K]
        g = np.ascontiguousarray(sh.transpose(1, 0, 2)).astype(bf16)  # [T,BS,K]
        in_maps.append({
            "g": g.reshape(T * BS, K),
            "af": af, "ab": ab, "p0": p0, "q0": q0,
        })

    nc = _get_module()
    res = run_bass_kernel_spmd(nc, in_maps, core_ids=list(range(NCORES)))

    lse_sum = lse.astype(np.float64).sum(axis=1)              # [B]
    logZ = np.empty(B, np.float64)
    for c in range(NCORES):
        r = res.results[c]
        dot = np.sum(r["pf"].astype(np.float64) * r["qb"].astype(np.float64),
                     axis=0)                                  # [BS]
        logZ[c * BS:(c + 1) * BS] = np.log(dot) + lse_sum[c * BS:(c + 1) * BS]
    return logZ.astype(np.float32)
